# revision 16
# baseline (speedup 1.0000x reference)
"""Multi-head attention (B=4, S=2048, D=1024, H=16) on 8 trn2 NeuronCores.

Sharding: batch x seq-half — core c owns batch b=c//2, query tokens
s in [1024*(c%2), 1024*(c%2)+1024), ALL 16 heads local; global token range
1024*c .. 1024*(c+1) so host assembly is a plain concat.

Per core, software-pipelined by head-pair (oc = output chunk = head pair):
  stage oc: project local-half K/V (1024 keys) and Q for head pair oc,
  AllGather K/V with the sibling core (pairwise, per 2-oc group), and run
  attention units for earlier head pairs concurrently — the ScalarE exp
  stream (the attention long pole) overlaps the projection matmuls.

Attention per (qw 512-query-window, head-pair): scores^T = K^T.T @ Q^T
(2-head row-packed matmuls via tile_position), exp on ScalarE (PSUM ->
SBUF bf16), PV with lhsT=[V|1] so PSUM row 64 accumulates the softmax
denominator. Normalize via DVE reciprocal + DRAM-bounce partition-
broadcast into g_sb; output projection straight from g_sb.

Key order per core is [pair-rank0 half, rank1 half] == global order (both
K and V), so softmax/PV are key-order consistent. Host folds the
1/sqrt(head_dim) scale into w_q and pre-tiles everything to bf16.

Biases are applied exactly on the host: b_v and b_o contribute
(b_v @ w_o.T + b_o) to every token (softmax rows sum to 1). b_q/b_k cannot
be folded; setup_inputs() generates them as zeros — a numpy fallback guards
the (never-exercised) nonzero case, as well as non-trivial masks.
"""

import numpy as np
import ml_dtypes

import concourse.bass as bass
import concourse.tile as tile
from concourse import mybir
from concourse.bass_utils import run_bass_kernel_spmd
from concourse.masks import make_identity

NCORES = 8
B, S, D, H = 4, 2048, 1024, 16
HD = D // H            # 64
P = 128
T = B * S              # 8192 tokens
TOK_PER_CORE = T // NCORES   # 1024
NCH = D // P           # 8 contraction chunks
NHP = H // 2           # 8 head pairs (= output-dim chunks)
SKT = S // P           # 16 key tiles per batch
LKT = 8                # local key tiles (1024 keys)
VROW = 2 * (HD + 1)    # 130 cols per k-tile in v_all ([V_h0|1|V_h1|1])
GCOL = 1024 + LKT * VROW   # 2064 gathered cols per oc (K then V)

BF16 = mybir.dt.bfloat16
F32 = mybir.dt.float32
bf16 = ml_dtypes.bfloat16

_CACHED_NC = None


def split_multi_waits(nc):
    """This walrus build supports one sync-wait per instruction; hoist extras
    onto same-engine NoOps inserted immediately before."""
    for f in nc.m.functions:
        for blk in f.blocks:
            insts = blk.instructions
            i = 0
            while i < len(insts):
                inst = insts[i]
                si = getattr(inst, "sync_info", None)
                if si is not None and si.on_wait and len(si.on_wait) > 1:
                    waits = list(si.on_wait)
                    for j, w in enumerate(waits[:-1]):
                        nop = mybir.InstNoOp(name=f"I-ws-{inst.name}-{j}",
                                             ins=[], outs=[])
                        nop.engine = inst.engine
                        nop.sync_info = mybir.SyncInfo(on_wait=[w], on_update=[])
                        insts.insert(i, nop)
                        i += 1
                    inst.sync_info = mybir.SyncInfo(on_wait=[waits[-1]],
                                                    on_update=si.on_update)
                i += 1


def build(split=True):
    global _CACHED_NC
    if split and _CACHED_NC is not None:
        return _CACHED_NC
    from contextlib import ExitStack

    nc = bass.Bass(num_devices=NCORES, target_bir_lowering=False, debug=False)

    # Inputs (per core). x*: local 1024 tokens as [ch, 128, 1024] of x^T.
    xq_d = nc.dram_tensor("xq", [NCH, P, 1024], BF16, kind="ExternalInput")
    xk_d = nc.dram_tensor("xk", [NCH, P, 1024], BF16, kind="ExternalInput")
    xv_d = nc.dram_tensor("xv", [NCH, P, 1024], BF16, kind="ExternalInput")
    # w[qkv]: [oc, 128, ch*128] of w.T (out-chunk major); wo: [hp, 128, 1024].
    wq_d = nc.dram_tensor("wq", [NCH, P, 1024], BF16, kind="ExternalInput")
    wk_d = nc.dram_tensor("wk", [NCH, P, 1024], BF16, kind="ExternalInput")
    wv_d = nc.dram_tensor("wv", [NCH, P, 1024], BF16, kind="ExternalInput")
    wo_d = nc.dram_tensor("wo", [NHP, P, 1024], BF16, kind="ExternalInput")
    out_d = nc.dram_tensor("out", [TOK_PER_CORE, D], F32, kind="ExternalOutput")

    # Internal DRAM bounces for the softmax-denominator reciprocal broadcast.
    rs_d = nc.dram_tensor("rs_d", [2 * NHP, 1024], F32)
    rcp_d = nc.dram_tensor("rcp_d", [2 * NHP, 1024], F32)
    # Tiny pair AllGather fired at t=0 to absorb CC-stream startup latency.
    warm_in = nc.dram_tensor("warm_in", [P, 8], BF16)
    warm_out = nc.dram_tensor("warm_out", [2, P, 8], BF16)
    # Pairwise K/V exchange, one AllGather per 2-oc group: local K^T cols +
    # V-natural (ones included) for both oc of the group.
    kvg_in = [nc.dram_tensor(f"kvg_in{g}", [P, 2 * GCOL], BF16)
              for g in range(4)]
    kvg_out = [nc.dram_tensor(f"kvg_out{g}", [2, P, 2 * GCOL], BF16)
               for g in range(4)]

    with tile.TileContext(nc, pool_alloc_mode="queue") as tc:
        with ExitStack() as ctx:
            const = ctx.enter_context(tc.tile_pool(name="const", bufs=1))
            persist = ctx.enter_context(tc.tile_pool(name="persist", bufs=1))
            wpool = ctx.enter_context(tc.tile_pool(name="wpool", bufs=2))
            work = ctx.enter_context(tc.tile_pool(name="work", bufs=2))
            expool = ctx.enter_context(tc.tile_pool(name="expool", bufs=4))
            npool = ctx.enter_context(tc.tile_pool(name="npool", bufs=2))
            psum = ctx.enter_context(tc.tile_pool(name="psum", bufs=2, space="PSUM"))

            nc.gpsimd.collective_compute(
                "AllGather", mybir.AluOpType.bypass,
                replica_groups=[[0, 1], [2, 3], [4, 5], [6, 7]],
                ins=[warm_in.ap()], outs=[warm_out.ap()],
            )

            ident = const.tile([P, P], BF16)
            make_identity(nc, ident)

            # Persistent SBUF (bytes/partition):
            kt_sb = persist.tile([P, NHP * S], BF16, tag="kt_sb")           # 32K
            v_all = persist.tile([P, NHP * SKT * VROW], BF16, tag="v_all")  # 32.5K
            qt_sb = persist.tile([P, NHP * 1024], BF16, tag="qt_sb")        # 16K
            wo_sb = persist.tile([P, NHP * 1024], BF16, tag="wo_sb")        # 16K
            g_sb0 = persist.tile([P, NHP * 512], BF16, tag="gsb0")          # 8K
            g_sb1 = persist.tile([P, NHP * 512], BF16, tag="gsb1")          # 8K
            g_sb = [g_sb0, g_sb1]
            # local-x staging, alive for the whole projection pipeline
            xq_sb = persist.tile([P, NCH * 1024], BF16, tag="xq_sb")        # 16K
            xk_sb = persist.tile([P, NCH * 1024], BF16, tag="xk_sb")        # 16K
            xv_sb = persist.tile([P, NCH * 1024], BF16, tag="xv_sb")        # 16K

            # ones columns of v_all (cols 64 and 129 of each 130-block)
            v_view = v_all[:].rearrange("p (n c) -> p n c", c=VROW)
            nc.vector.memset(v_view[:, :, HD], 1.0)
            nc.vector.memset(v_view[:, :, 2 * HD + 1], 1.0)

            for ch in range(NCH):
                nc.sync.dma_start(xk_sb[:, ch * 1024:(ch + 1) * 1024],
                                  xk_d.ap()[ch])
            for ch in range(NCH):
                nc.sync.dma_start(xv_sb[:, ch * 1024:(ch + 1) * 1024],
                                  xv_d.ap()[ch])
            for ch in range(NCH):
                nc.sync.dma_start(xq_sb[:, ch * 1024:(ch + 1) * 1024],
                                  xq_d.ap()[ch])

            def proj_mm(w_sb, x_sb, tc_):
                ps = psum.tile([P, 1024], F32, tag="sc")
                for ch in range(NCH):
                    nc.tensor.matmul(
                        ps[:, 0:512],
                        w_sb[:, ch * P:(ch + 1) * P],
                        x_sb[:, ch * 1024 + tc_ * 512: ch * 1024 + (tc_ + 1) * 512],
                        start=(ch == 0), stop=(ch == NCH - 1))
                return ps

            # ---- full projection work for one head pair oc ----
            def proj_oc(oc):
                wkc = wpool.tile([P, 1024], BF16, tag="wk")
                nc.sync.dma_start(wkc[:], wk_d.ap()[oc])
                wvc = wpool.tile([P, 1024], BF16, tag="wv")
                nc.sync.dma_start(wvc[:], wv_d.ap()[oc])
                wqc = wpool.tile([P, 1024], BF16, tag="wq")
                nc.sync.dma_start(wqc[:], wq_d.ap()[oc])
                for tc_ in range(2):  # K local 1024 keys -> kt_sb slot 0
                    ps = proj_mm(wkc, xk_sb, tc_)
                    col = oc * S + tc_ * 512
                    nc.vector.tensor_copy(kt_sb[:, col:col + 512], ps[:, 0:512])
                for tc_ in range(2):  # V local -> v_all kt slots 0-7
                    ps = proj_mm(wvc, xv_sb, tc_)
                    vt_scr = work.tile([P, 512], BF16, tag="vt_scr")
                    nc.vector.tensor_copy(vt_scr[:], ps[:, 0:512])
                    tp4 = ps[:, 512:768].bitcast(BF16)
                    for j in range(4):
                        tp = tp4[:, j * P:(j + 1) * P]
                        nc.tensor.transpose(
                            tp[:], vt_scr[:, j * P:(j + 1) * P], ident[:])
                        kt = tc_ * 4 + j
                        base = (oc * SKT + kt) * VROW
                        nc.vector.tensor_copy(
                            v_all[:, base:base + VROW]
                            .rearrange("p (b c) -> p b c", c=HD + 1)[:, :, 0:HD],
                            tp[:].rearrange("p (b c) -> p b c", c=HD))
                for tc_ in range(2):  # Q local -> qt_sb
                    ps = proj_mm(wqc, xq_sb, tc_)
                    col = oc * 1024 + tc_ * 512
                    nc.vector.tensor_copy(qt_sb[:, col:col + 512], ps[:, 0:512])

            # ---- pairwise K/V gather for oc group g = {2g, 2g+1} ----
            def gather(g):
                for i, oc in enumerate((2 * g, 2 * g + 1)):
                    gbase = i * GCOL
                    # local K cols (slot 0 of oc) -> kvg_in
                    nc.gpsimd.dma_start(
                        kvg_in[g].ap()[:, gbase:gbase + 1024],
                        kt_sb[:, oc * S: oc * S + 1024])
                    # local V region (kt 0-7 of oc, ones included) -> kvg_in
                    nc.gpsimd.dma_start(
                        kvg_in[g].ap()[:, gbase + 1024:gbase + GCOL],
                        v_all[:, oc * SKT * VROW: (oc * SKT + LKT) * VROW])
                nc.gpsimd.collective_compute(
                    "AllGather", mybir.AluOpType.bypass,
                    replica_groups=[[0, 1], [2, 3], [4, 5], [6, 7]],
                    ins=[kvg_in[g].ap()], outs=[kvg_out[g].ap()],
                )
                for h in range(2):      # both pair ranks -> global key order
                    for i, oc in enumerate((2 * g, 2 * g + 1)):
                        gbase = i * GCOL
                        nc.gpsimd.dma_start(
                            kt_sb[:, oc * S + h * 1024: oc * S + (h + 1) * 1024],
                            kvg_out[g].ap()[h][:, gbase:gbase + 1024])
                        nc.gpsimd.dma_start(
                            v_all[:, (oc * SKT + h * LKT) * VROW:
                                  (oc * SKT + (h + 1) * LKT) * VROW],
                            kvg_out[g].ap()[h][:, gbase + 1024:gbase + GCOL])

            # ---- attention unit: one (head pair, 512-query window) ----
            def attn_unit(qw, hp):
                unit = qw * NHP + hp
                qcol = hp * 1024 + qw * 512
                pv0 = psum.tile([HD + 1, 512], F32, tag="pv0")
                pv1 = psum.tile([HD + 1, 512], F32, tag="pv1")
                for kt in range(SKT):
                    kcol = hp * S + kt * P
                    sc = psum.tile([P, 1024], F32, tag="sc")
                    nc.tensor.matmul(
                        sc[:, 0:512],
                        kt_sb[0:HD, kcol:kcol + P],
                        qt_sb[0:HD, qcol:qcol + 512],
                        start=True, stop=True, tile_position=(0, 0))
                    nc.tensor.matmul(
                        sc[:, 512:1024],
                        kt_sb[HD:2 * HD, kcol:kcol + P],
                        qt_sb[HD:2 * HD, qcol:qcol + 512],
                        start=True, stop=True, tile_position=(HD, 0))
                    ex = expool.tile([P, 1024], BF16, tag="ex")
                    nc.scalar.activation(
                        ex[:], sc[:], mybir.ActivationFunctionType.Exp)
                    vb = (hp * SKT + kt) * VROW
                    nc.tensor.matmul(
                        pv0[:], v_all[:, vb:vb + HD + 1],
                        ex[:, 0:512],
                        start=(kt == 0), stop=(kt == SKT - 1))
                    nc.tensor.matmul(
                        pv1[:], v_all[:, vb + HD + 1:vb + VROW],
                        ex[:, 512:1024],
                        start=(kt == 0), stop=(kt == SKT - 1))
                # normalize: rowsum rows (PSUM row 64) -> DRAM -> [128,8]
                # reciprocal (128-lane parallel) -> DRAM -> partition-broadcast
                # -> multiply into g_sb.
                rs = npool.tile([HD + 1, 1024], F32, tag="rs")
                nc.vector.tensor_copy(rs[HD:HD + 1, 0:512], pv0[HD:HD + 1, :])
                nc.vector.tensor_copy(rs[HD:HD + 1, 512:1024], pv1[HD:HD + 1, :])
                nc.gpsimd.dma_start(
                    rs_d.ap()[unit].rearrange("(a f) -> a f", a=1),
                    rs[HD:HD + 1, :])
                rsw = npool.tile([P, 8], F32, tag="rsw")
                nc.gpsimd.dma_start(
                    rsw[:], rs_d.ap()[unit].rearrange("(p f) -> p f", f=8))
                rcw = npool.tile([P, 8], F32, tag="rcw")
                nc.vector.reciprocal(rcw[:], rsw[:])
                nc.gpsimd.dma_start(
                    rcp_d.ap()[unit].rearrange("(p f) -> p f", f=8),
                    rcw[:])
                bc0 = npool.tile([HD, 512], F32, tag="bc0")
                bc1 = npool.tile([HD, 512], F32, tag="bc1")
                nc.gpsimd.dma_start(
                    bc0[:], rcp_d.ap()[unit].rearrange("(a f) -> a f", a=1)[:, 0:512].to_broadcast((HD, 512)))
                nc.gpsimd.dma_start(
                    bc1[:], rcp_d.ap()[unit].rearrange("(a f) -> a f", a=1)[:, 512:1024].to_broadcast((HD, 512)))
                # head0 lands directly in g_sb rows 0:64; head1 via a bounce
                # tile + SBUF->SBUF DMA into rows 64:128.
                nc.vector.tensor_mul(g_sb[qw][0:HD, hp * 512:(hp + 1) * 512],
                                     pv0[0:HD, :], bc0[:])
                at1 = npool.tile([HD, 512], BF16, tag="at1")
                nc.vector.tensor_mul(at1[:], pv1[0:HD, :], bc1[:])
                nc.sync.dma_start(g_sb[qw][HD:2 * HD, hp * 512:(hp + 1) * 512],
                                  at1[:])

            # ---- one output-projection chunk (128 tokens x 512 dims) ----
            def outproj_chunk(qw, t128, dh):
                po = psum.tile([P, 512], F32, tag="pv0")
                for hp in range(NHP):
                    nc.tensor.matmul(
                        po[:, 0:512],
                        g_sb[qw][:, hp * 512 + t128 * P: hp * 512 + (t128 + 1) * P],
                        wo_sb[:, hp * 1024 + dh * 512: hp * 1024 + (dh + 1) * 512],
                        start=(hp == 0), stop=(hp == NHP - 1))
                osb = work.tile([P, 512], F32, tag="osb")
                nc.vector.tensor_copy(osb[:], po[:])
                row = qw * 512 + t128 * P
                nc.sync.dma_start(
                    out_d.ap()[row:row + P, dh * 512:(dh + 1) * 512],
                    osb[:])

            # ---- schedule: front-load 6 proj stages + 3 gathers — the PE
            # chews projections for ~65us, exactly covering the CC-stream
            # init + first-gather latency, so the first attention unit starts
            # with everything warm; only proj 6/7 interleave with units, so
            # 14 of 16 units run at the pure ScalarE-exp (ACT) pace.
            proj_oc(0)
            proj_oc(1)
            gather(0)
            proj_oc(2)
            proj_oc(3)
            gather(1)
            proj_oc(4)
            proj_oc(5)
            gather(2)
            for w_hp in range(NHP):
                nc.sync.dma_start(wo_sb[:, w_hp * 1024:(w_hp + 1) * 1024],
                                  wo_d.ap()[w_hp])
            attn_unit(0, 0)
            proj_oc(6)
            attn_unit(0, 1)
            proj_oc(7)
            gather(3)
            for hp in range(2, NHP):
                attn_unit(0, hp)
            for hp in range(NHP):
                attn_unit(1, hp)
                outproj_chunk(0, hp // 2, hp % 2)
            for t128 in range(4):
                for dh in range(2):
                    outproj_chunk(1, t128, dh)

    if split:
        split_multi_waits(nc)
        _CACHED_NC = nc
    return nc


def _host_prep(query, key, value, w_q, w_k, w_v, w_o):
    sc = 1.0 / np.sqrt(np.float32(HD))

    def tile_x(x):  # [1024, D] -> [NCH, 128, 1024] bf16 of x^T
        xt = np.asarray(x, np.float32).T.reshape(NCH, P, 1024)
        return np.ascontiguousarray(xt.astype(bf16))

    def tile_w(w, scale=1.0):  # [D,D] -> [oc, 128, ch*128] of w.T
        wt = (np.asarray(w, np.float32) * scale).T             # [in D, out D]
        wt = wt.reshape(NCH, P, NCH, P).transpose(2, 1, 0, 3)  # [oc, p, ch, f]
        return np.ascontiguousarray(wt.reshape(NCH, P, 1024).astype(bf16))

    wq_t, wk_t, wv_t = tile_w(w_q, sc), tile_w(w_k), tile_w(w_v)
    wo_t = np.ascontiguousarray(
        np.asarray(w_o, np.float32).T.reshape(NHP, P, 1024).astype(bf16))

    q3 = np.asarray(query, np.float32).reshape(B, S, D)
    k3 = np.asarray(key, np.float32).reshape(B, S, D)
    v3 = np.asarray(value, np.float32).reshape(B, S, D)

    in_maps = []
    for c in range(NCORES):
        b, half = c // 2, c % 2
        sl = slice(half * 1024, (half + 1) * 1024)
        in_maps.append({
            "xq": tile_x(q3[b, sl]),
            "xk": tile_x(k3[b, sl]),
            "xv": tile_x(v3[b, sl]),
            "wq": wq_t, "wk": wk_t, "wv": wv_t, "wo": wo_t,
        })
    return in_maps


def _numpy_fallback(query, key, value, attn_mask, key_padding_mask,
                    w_q, b_q, w_k, b_k, w_v, b_v, w_o, b_o):
    q = query.reshape(T, D) @ w_q.T + b_q
    k = key.reshape(T, D) @ w_k.T + b_k
    v = value.reshape(T, D) @ w_v.T + b_v
    qh = q.reshape(B, S, H, HD).transpose(0, 2, 1, 3)
    kh = k.reshape(B, S, H, HD).transpose(0, 2, 1, 3)
    vh = v.reshape(B, S, H, HD).transpose(0, 2, 1, 3)
    out = np.empty((B, H, S, HD), np.float32)
    neg = np.finfo(np.float32).min
    for b in range(B):
        for h in range(H):
            s = (qh[b, h] @ kh[b, h].T) / np.sqrt(np.float32(HD))
            s = np.where(attn_mask, s, neg)
            s = np.where(key_padding_mask[b][None, :], s, neg)
            s = s - s.max(axis=-1, keepdims=True)
            e = np.exp(s)
            a = e / e.sum(axis=-1, keepdims=True)
            out[b, h] = a @ vh[b, h]
    o = out.transpose(0, 2, 1, 3).reshape(T, D)
    return (o @ w_o.T + b_o).reshape(B, S, D).astype(np.float32)


def kernel(query, key, value, attn_mask, key_padding_mask,
           w_q, b_q, w_k, b_k, w_v, b_v, w_o, b_o):
    query = np.asarray(query, np.float32)
    key = np.asarray(key, np.float32)
    value = np.asarray(value, np.float32)
    attn_mask = np.asarray(attn_mask)
    key_padding_mask = np.asarray(key_padding_mask)
    w_q, b_q = np.asarray(w_q, np.float32), np.asarray(b_q, np.float32)
    w_k, b_k = np.asarray(w_k, np.float32), np.asarray(b_k, np.float32)
    w_v, b_v = np.asarray(w_v, np.float32), np.asarray(b_v, np.float32)
    w_o, b_o = np.asarray(w_o, np.float32), np.asarray(b_o, np.float32)

    if (not attn_mask.all() or not key_padding_mask.all()
            or b_q.any() or b_k.any()):
        return _numpy_fallback(query, key, value, attn_mask, key_padding_mask,
                               w_q, b_q, w_k, b_k, w_v, b_v, w_o, b_o)

    nc = build()
    in_maps = _host_prep(query, key, value, w_q, w_k, w_v, w_o)
    res = run_bass_kernel_spmd(nc, in_maps, list(range(NCORES)))

    out = np.empty((T, D), np.float32)
    for c in range(NCORES):
        out[TOK_PER_CORE * c:TOK_PER_CORE * (c + 1)] = \
            res.results[c]["out"].reshape(TOK_PER_CORE, D)
    # exact host-side bias fold: softmax rows sum to 1 => + (b_v @ w_o.T + b_o)
    out += b_v @ w_o.T + b_o
    return out.reshape(B, S, D)


# revision 19
# speedup vs baseline: 1.0170x; 1.0170x over previous
"""Multi-head attention (B=4, S=2048, D=1024, H=16) on 8 trn2 NeuronCores.

Sharding: batch x seq-half — core c owns batch b=c//2, query tokens
s in [1024*(c%2), 1024*(c%2)+1024), ALL 16 heads local; global token range
1024*c .. 1024*(c+1) so host assembly is a plain concat.

Per core, software-pipelined by head-pair (oc = output chunk = head pair):
  stage oc: project local-half K/V (1024 keys) and Q for head pair oc,
  AllGather K/V with the sibling core (pairwise, per 2-oc group), and run
  attention units for earlier head pairs concurrently — the ScalarE exp
  stream (the attention long pole) overlaps the projection matmuls.

Attention per (qw 512-query-window, head-pair): scores^T = K^T.T @ Q^T
(2-head row-packed matmuls via tile_position), exp on ScalarE (PSUM ->
SBUF bf16), PV with lhsT=[V|1] so PSUM row 64 accumulates the softmax
denominator. Normalize via DVE reciprocal + DRAM-bounce partition-
broadcast into g_sb; output projection straight from g_sb.

Key order per core is [pair-rank0 half, rank1 half] == global order (both
K and V), so softmax/PV are key-order consistent. Host folds the
1/sqrt(head_dim) scale into w_q and pre-tiles everything to bf16.

Biases are applied exactly on the host: b_v and b_o contribute
(b_v @ w_o.T + b_o) to every token (softmax rows sum to 1). b_q/b_k cannot
be folded; setup_inputs() generates them as zeros — a numpy fallback guards
the (never-exercised) nonzero case, as well as non-trivial masks.
"""

import numpy as np
import ml_dtypes

import concourse.bass as bass
import concourse.tile as tile
from concourse import mybir
from concourse.bass_utils import run_bass_kernel_spmd
from concourse.masks import make_identity

NCORES = 8
B, S, D, H = 4, 2048, 1024, 16
HD = D // H            # 64
P = 128
T = B * S              # 8192 tokens
TOK_PER_CORE = T // NCORES   # 1024
NCH = D // P           # 8 contraction chunks
NHP = H // 2           # 8 head pairs (= output-dim chunks)
SKT = S // P           # 16 key tiles per batch
LKT = 8                # local key tiles (1024 keys)
VROW = 2 * (HD + 1)    # 130 cols per k-tile in v_all ([V_h0|1|V_h1|1])
GCOL = 1024 + LKT * VROW   # 2064 gathered cols per oc (K then V)

BF16 = mybir.dt.bfloat16
F32 = mybir.dt.float32
bf16 = ml_dtypes.bfloat16

_CACHED_NC = None


def split_multi_waits(nc):
    """This walrus build supports one sync-wait per instruction; hoist extras
    onto same-engine NoOps inserted immediately before."""
    for f in nc.m.functions:
        for blk in f.blocks:
            insts = blk.instructions
            i = 0
            while i < len(insts):
                inst = insts[i]
                si = getattr(inst, "sync_info", None)
                if si is not None and si.on_wait and len(si.on_wait) > 1:
                    waits = list(si.on_wait)
                    for j, w in enumerate(waits[:-1]):
                        nop = mybir.InstNoOp(name=f"I-ws-{inst.name}-{j}",
                                             ins=[], outs=[])
                        nop.engine = inst.engine
                        nop.sync_info = mybir.SyncInfo(on_wait=[w], on_update=[])
                        insts.insert(i, nop)
                        i += 1
                    inst.sync_info = mybir.SyncInfo(on_wait=[waits[-1]],
                                                    on_update=si.on_update)
                i += 1


def build(split=True):
    global _CACHED_NC
    if split and _CACHED_NC is not None:
        return _CACHED_NC
    from contextlib import ExitStack

    nc = bass.Bass(num_devices=NCORES, target_bir_lowering=False, debug=False)

    # Inputs (per core). x*: local 1024 tokens as [ch, 128, 1024] of x^T.
    xq_d = nc.dram_tensor("xq", [NCH, P, 1024], BF16, kind="ExternalInput")
    xk_d = nc.dram_tensor("xk", [NCH, P, 1024], BF16, kind="ExternalInput")
    xv_d = nc.dram_tensor("xv", [NCH, P, 1024], BF16, kind="ExternalInput")
    # w[qkv]: [oc, 128, ch*128] of w.T (out-chunk major); wo: [hp, 128, 1024].
    wq_d = nc.dram_tensor("wq", [NCH, P, 1024], BF16, kind="ExternalInput")
    wk_d = nc.dram_tensor("wk", [NCH, P, 1024], BF16, kind="ExternalInput")
    wv_d = nc.dram_tensor("wv", [NCH, P, 1024], BF16, kind="ExternalInput")
    wo_d = nc.dram_tensor("wo", [NHP, P, 1024], BF16, kind="ExternalInput")
    out_d = nc.dram_tensor("out", [TOK_PER_CORE, D], F32, kind="ExternalOutput")

    # Internal DRAM bounces for the softmax-denominator reciprocal broadcast.
    rs_d = nc.dram_tensor("rs_d", [2 * NHP, 1024], F32)
    rcp_d = nc.dram_tensor("rcp_d", [2 * NHP, 1024], F32)
    # Tiny pair AllGather fired at t=0 to absorb CC-stream startup latency.
    warm_in = nc.dram_tensor("warm_in", [P, 8], BF16)
    warm_out = nc.dram_tensor("warm_out", [2, P, 8], BF16)
    # Pairwise K/V exchange, one AllGather per 2-oc group: local K^T cols +
    # V-natural (ones included) for both oc of the group.
    kvg_in = [nc.dram_tensor(f"kvg_in{g}", [P, 2 * GCOL], BF16)
              for g in range(4)]
    kvg_out = [nc.dram_tensor(f"kvg_out{g}", [2, P, 2 * GCOL], BF16)
               for g in range(4)]

    with tile.TileContext(nc, pool_alloc_mode="queue") as tc:
        with ExitStack() as ctx:
            const = ctx.enter_context(tc.tile_pool(name="const", bufs=1))
            persist = ctx.enter_context(tc.tile_pool(name="persist", bufs=1))
            wpool = ctx.enter_context(tc.tile_pool(name="wpool", bufs=2))
            work = ctx.enter_context(tc.tile_pool(name="work", bufs=2))
            expool = ctx.enter_context(tc.tile_pool(name="expool", bufs=4))
            npool = ctx.enter_context(tc.tile_pool(name="npool", bufs=2))
            psum = ctx.enter_context(tc.tile_pool(name="psum", bufs=2, space="PSUM"))

            nc.gpsimd.collective_compute(
                "AllGather", mybir.AluOpType.bypass,
                replica_groups=[[0, 1], [2, 3], [4, 5], [6, 7]],
                ins=[warm_in.ap()], outs=[warm_out.ap()],
            )

            ident = const.tile([P, P], BF16)
            make_identity(nc, ident)

            # Persistent SBUF (bytes/partition):
            kt_sb = persist.tile([P, NHP * S], BF16, tag="kt_sb")           # 32K
            v_all = persist.tile([P, NHP * SKT * VROW], BF16, tag="v_all")  # 32.5K
            qt_sb = persist.tile([P, NHP * 1024], BF16, tag="qt_sb")        # 16K
            wo_sb = persist.tile([P, NHP * 1024], BF16, tag="wo_sb")        # 16K
            g_sb0 = persist.tile([P, NHP * 512], BF16, tag="gsb0")          # 8K
            g_sb1 = persist.tile([P, NHP * 512], BF16, tag="gsb1")          # 8K
            g_sb = [g_sb0, g_sb1]
            # local-x staging, alive for the whole projection pipeline
            xq_sb = persist.tile([P, NCH * 1024], BF16, tag="xq_sb")        # 16K
            xk_sb = persist.tile([P, NCH * 1024], BF16, tag="xk_sb")        # 16K
            xv_sb = persist.tile([P, NCH * 1024], BF16, tag="xv_sb")        # 16K

            # ones columns of v_all (cols 64 and 129 of each 130-block)
            v_view = v_all[:].rearrange("p (n c) -> p n c", c=VROW)
            nc.vector.memset(v_view[:, :, HD], 1.0)
            nc.vector.memset(v_view[:, :, 2 * HD + 1], 1.0)

            for ch in range(NCH):
                nc.sync.dma_start(xk_sb[:, ch * 1024:(ch + 1) * 1024],
                                  xk_d.ap()[ch])
            for ch in range(NCH):
                nc.sync.dma_start(xv_sb[:, ch * 1024:(ch + 1) * 1024],
                                  xv_d.ap()[ch])
            for ch in range(NCH):
                nc.sync.dma_start(xq_sb[:, ch * 1024:(ch + 1) * 1024],
                                  xq_d.ap()[ch])

            def proj_mm(w_sb, x_sb, tc_):
                ps = psum.tile([P, 1024], F32, tag="sc")
                for ch in range(NCH):
                    nc.tensor.matmul(
                        ps[:, 0:512],
                        w_sb[:, ch * P:(ch + 1) * P],
                        x_sb[:, ch * 1024 + tc_ * 512: ch * 1024 + (tc_ + 1) * 512],
                        start=(ch == 0), stop=(ch == NCH - 1))
                return ps

            # ---- K/V projection for one head pair oc (gather-critical) ----
            def kv_oc(oc):
                wkc = wpool.tile([P, 1024], BF16, tag="wk")
                nc.sync.dma_start(wkc[:], wk_d.ap()[oc])
                wvc = wpool.tile([P, 1024], BF16, tag="wv")
                nc.sync.dma_start(wvc[:], wv_d.ap()[oc])
                for tc_ in range(2):  # K local 1024 keys -> kt_sb slot 0
                    ps = proj_mm(wkc, xk_sb, tc_)
                    col = oc * S + tc_ * 512
                    nc.vector.tensor_copy(kt_sb[:, col:col + 512], ps[:, 0:512])
                for tc_ in range(2):  # V local -> v_all kt slots 0-7
                    ps = proj_mm(wvc, xv_sb, tc_)
                    vt_scr = work.tile([P, 512], BF16, tag="vt_scr")
                    nc.vector.tensor_copy(vt_scr[:], ps[:, 0:512])
                    tp4 = ps[:, 512:768].bitcast(BF16)
                    for j in range(4):
                        tp = tp4[:, j * P:(j + 1) * P]
                        nc.tensor.transpose(
                            tp[:], vt_scr[:, j * P:(j + 1) * P], ident[:])
                        kt = tc_ * 4 + j
                        base = (oc * SKT + kt) * VROW
                        nc.vector.tensor_copy(
                            v_all[:, base:base + VROW]
                            .rearrange("p (b c) -> p b c", c=HD + 1)[:, :, 0:HD],
                            tp[:].rearrange("p (b c) -> p b c", c=HD))

            # ---- Q projection for one head pair (off the gather path) ----
            def q_oc(oc):
                wqc = wpool.tile([P, 1024], BF16, tag="wq")
                nc.sync.dma_start(wqc[:], wq_d.ap()[oc])
                for tc_ in range(2):  # Q local -> qt_sb
                    ps = proj_mm(wqc, xq_sb, tc_)
                    col = oc * 1024 + tc_ * 512
                    nc.vector.tensor_copy(qt_sb[:, col:col + 512], ps[:, 0:512])

            # ---- pairwise K/V gather for oc group g = {2g, 2g+1} ----
            def gather(g):
                for i, oc in enumerate((2 * g, 2 * g + 1)):
                    gbase = i * GCOL
                    # local K cols (slot 0 of oc) -> kvg_in
                    nc.gpsimd.dma_start(
                        kvg_in[g].ap()[:, gbase:gbase + 1024],
                        kt_sb[:, oc * S: oc * S + 1024])
                    # local V region (kt 0-7 of oc, ones included) -> kvg_in
                    nc.gpsimd.dma_start(
                        kvg_in[g].ap()[:, gbase + 1024:gbase + GCOL],
                        v_all[:, oc * SKT * VROW: (oc * SKT + LKT) * VROW])
                nc.gpsimd.collective_compute(
                    "AllGather", mybir.AluOpType.bypass,
                    replica_groups=[[0, 1], [2, 3], [4, 5], [6, 7]],
                    ins=[kvg_in[g].ap()], outs=[kvg_out[g].ap()],
                )
                for h in range(2):      # both pair ranks -> global key order
                    for i, oc in enumerate((2 * g, 2 * g + 1)):
                        gbase = i * GCOL
                        nc.gpsimd.dma_start(
                            kt_sb[:, oc * S + h * 1024: oc * S + (h + 1) * 1024],
                            kvg_out[g].ap()[h][:, gbase:gbase + 1024])
                        nc.gpsimd.dma_start(
                            v_all[:, (oc * SKT + h * LKT) * VROW:
                                  (oc * SKT + (h + 1) * LKT) * VROW],
                            kvg_out[g].ap()[h][:, gbase + 1024:gbase + GCOL])

            # ---- attention unit: one (head pair, 512-query window) ----
            def attn_unit(qw, hp):
                unit = qw * NHP + hp
                qcol = hp * 1024 + qw * 512
                pv0 = psum.tile([HD + 1, 512], F32, tag="pv0")
                pv1 = psum.tile([HD + 1, 512], F32, tag="pv1")
                for kt in range(SKT):
                    kcol = hp * S + kt * P
                    sc = psum.tile([P, 1024], F32, tag="sc")
                    nc.tensor.matmul(
                        sc[:, 0:512],
                        kt_sb[0:HD, kcol:kcol + P],
                        qt_sb[0:HD, qcol:qcol + 512],
                        start=True, stop=True, tile_position=(0, 0))
                    nc.tensor.matmul(
                        sc[:, 512:1024],
                        kt_sb[HD:2 * HD, kcol:kcol + P],
                        qt_sb[HD:2 * HD, qcol:qcol + 512],
                        start=True, stop=True, tile_position=(HD, 0))
                    ex = expool.tile([P, 1024], BF16, tag="ex")
                    nc.scalar.activation(
                        ex[:], sc[:], mybir.ActivationFunctionType.Exp)
                    vb = (hp * SKT + kt) * VROW
                    nc.tensor.matmul(
                        pv0[:], v_all[:, vb:vb + HD + 1],
                        ex[:, 0:512],
                        start=(kt == 0), stop=(kt == SKT - 1))
                    nc.tensor.matmul(
                        pv1[:], v_all[:, vb + HD + 1:vb + VROW],
                        ex[:, 512:1024],
                        start=(kt == 0), stop=(kt == SKT - 1))
                # normalize: rowsum rows (PSUM row 64) -> DRAM -> [128,8]
                # reciprocal (128-lane parallel) -> DRAM -> partition-broadcast
                # -> multiply into g_sb.
                rs = npool.tile([HD + 1, 1024], F32, tag="rs")
                nc.vector.tensor_copy(rs[HD:HD + 1, 0:512], pv0[HD:HD + 1, :])
                nc.vector.tensor_copy(rs[HD:HD + 1, 512:1024], pv1[HD:HD + 1, :])
                nc.gpsimd.dma_start(
                    rs_d.ap()[unit].rearrange("(a f) -> a f", a=1),
                    rs[HD:HD + 1, :])
                rsw = npool.tile([P, 8], F32, tag="rsw")
                nc.gpsimd.dma_start(
                    rsw[:], rs_d.ap()[unit].rearrange("(p f) -> p f", f=8))
                rcw = npool.tile([P, 8], F32, tag="rcw")
                nc.vector.reciprocal(rcw[:], rsw[:])
                nc.gpsimd.dma_start(
                    rcp_d.ap()[unit].rearrange("(p f) -> p f", f=8),
                    rcw[:])
                bc0 = npool.tile([HD, 512], F32, tag="bc0")
                bc1 = npool.tile([HD, 512], F32, tag="bc1")
                nc.gpsimd.dma_start(
                    bc0[:], rcp_d.ap()[unit].rearrange("(a f) -> a f", a=1)[:, 0:512].to_broadcast((HD, 512)))
                nc.gpsimd.dma_start(
                    bc1[:], rcp_d.ap()[unit].rearrange("(a f) -> a f", a=1)[:, 512:1024].to_broadcast((HD, 512)))
                # head0 lands directly in g_sb rows 0:64; head1 via a bounce
                # tile + SBUF->SBUF DMA into rows 64:128.
                nc.vector.tensor_mul(g_sb[qw][0:HD, hp * 512:(hp + 1) * 512],
                                     pv0[0:HD, :], bc0[:])
                at1 = npool.tile([HD, 512], BF16, tag="at1")
                nc.vector.tensor_mul(at1[:], pv1[0:HD, :], bc1[:])
                nc.sync.dma_start(g_sb[qw][HD:2 * HD, hp * 512:(hp + 1) * 512],
                                  at1[:])

            # ---- one output-projection chunk (128 tokens x 512 dims) ----
            def outproj_chunk(qw, t128, dh):
                po = psum.tile([P, 512], F32, tag="pv0")
                for hp in range(NHP):
                    nc.tensor.matmul(
                        po[:, 0:512],
                        g_sb[qw][:, hp * 512 + t128 * P: hp * 512 + (t128 + 1) * P],
                        wo_sb[:, hp * 1024 + dh * 512: hp * 1024 + (dh + 1) * 512],
                        start=(hp == 0), stop=(hp == NHP - 1))
                osb = work.tile([P, 512], F32, tag="osb")
                nc.vector.tensor_copy(osb[:], po[:])
                row = qw * 512 + t128 * P
                nc.sync.dma_start(
                    out_d.ap()[row:row + P, dh * 512:(dh + 1) * 512],
                    osb[:])

            # ---- schedule: oc-major pipeline; K/V + gather staging lead
            # each pair, Q projections trail (off the gather critical path) --
            kv_oc(0)
            kv_oc(1)
            gather(0)
            q_oc(0)
            q_oc(1)
            kv_oc(2)
            kv_oc(3)
            gather(1)
            q_oc(2)
            q_oc(3)
            for hp in range(NHP):
                nxt = hp + 4
                if nxt < NCH:
                    kv_oc(nxt)
                    if nxt % 2 == 1:
                        gather(nxt // 2)
                        q_oc(nxt - 1)
                        q_oc(nxt)
                if hp == 0:
                    for w_hp in range(NHP):
                        nc.sync.dma_start(
                            wo_sb[:, w_hp * 1024:(w_hp + 1) * 1024],
                            wo_d.ap()[w_hp])
                attn_unit(0, hp)
            for hp in range(NHP):
                attn_unit(1, hp)
                outproj_chunk(0, hp // 2, hp % 2)
            for t128 in range(4):
                for dh in range(2):
                    outproj_chunk(1, t128, dh)

    if split:
        split_multi_waits(nc)
        _CACHED_NC = nc
    return nc


def _host_prep(query, key, value, w_q, w_k, w_v, w_o):
    sc = 1.0 / np.sqrt(np.float32(HD))

    def tile_x(x):  # [1024, D] -> [NCH, 128, 1024] bf16 of x^T
        xt = np.asarray(x, np.float32).T.reshape(NCH, P, 1024)
        return np.ascontiguousarray(xt.astype(bf16))

    def tile_w(w, scale=1.0):  # [D,D] -> [oc, 128, ch*128] of w.T
        wt = (np.asarray(w, np.float32) * scale).T             # [in D, out D]
        wt = wt.reshape(NCH, P, NCH, P).transpose(2, 1, 0, 3)  # [oc, p, ch, f]
        return np.ascontiguousarray(wt.reshape(NCH, P, 1024).astype(bf16))

    wq_t, wk_t, wv_t = tile_w(w_q, sc), tile_w(w_k), tile_w(w_v)
    wo_t = np.ascontiguousarray(
        np.asarray(w_o, np.float32).T.reshape(NHP, P, 1024).astype(bf16))

    q3 = np.asarray(query, np.float32).reshape(B, S, D)
    k3 = np.asarray(key, np.float32).reshape(B, S, D)
    v3 = np.asarray(value, np.float32).reshape(B, S, D)

    in_maps = []
    for c in range(NCORES):
        b, half = c // 2, c % 2
        sl = slice(half * 1024, (half + 1) * 1024)
        in_maps.append({
            "xq": tile_x(q3[b, sl]),
            "xk": tile_x(k3[b, sl]),
            "xv": tile_x(v3[b, sl]),
            "wq": wq_t, "wk": wk_t, "wv": wv_t, "wo": wo_t,
        })
    return in_maps


def _numpy_fallback(query, key, value, attn_mask, key_padding_mask,
                    w_q, b_q, w_k, b_k, w_v, b_v, w_o, b_o):
    q = query.reshape(T, D) @ w_q.T + b_q
    k = key.reshape(T, D) @ w_k.T + b_k
    v = value.reshape(T, D) @ w_v.T + b_v
    qh = q.reshape(B, S, H, HD).transpose(0, 2, 1, 3)
    kh = k.reshape(B, S, H, HD).transpose(0, 2, 1, 3)
    vh = v.reshape(B, S, H, HD).transpose(0, 2, 1, 3)
    out = np.empty((B, H, S, HD), np.float32)
    neg = np.finfo(np.float32).min
    for b in range(B):
        for h in range(H):
            s = (qh[b, h] @ kh[b, h].T) / np.sqrt(np.float32(HD))
            s = np.where(attn_mask, s, neg)
            s = np.where(key_padding_mask[b][None, :], s, neg)
            s = s - s.max(axis=-1, keepdims=True)
            e = np.exp(s)
            a = e / e.sum(axis=-1, keepdims=True)
            out[b, h] = a @ vh[b, h]
    o = out.transpose(0, 2, 1, 3).reshape(T, D)
    return (o @ w_o.T + b_o).reshape(B, S, D).astype(np.float32)


def kernel(query, key, value, attn_mask, key_padding_mask,
           w_q, b_q, w_k, b_k, w_v, b_v, w_o, b_o):
    query = np.asarray(query, np.float32)
    key = np.asarray(key, np.float32)
    value = np.asarray(value, np.float32)
    attn_mask = np.asarray(attn_mask)
    key_padding_mask = np.asarray(key_padding_mask)
    w_q, b_q = np.asarray(w_q, np.float32), np.asarray(b_q, np.float32)
    w_k, b_k = np.asarray(w_k, np.float32), np.asarray(b_k, np.float32)
    w_v, b_v = np.asarray(w_v, np.float32), np.asarray(b_v, np.float32)
    w_o, b_o = np.asarray(w_o, np.float32), np.asarray(b_o, np.float32)

    if (not attn_mask.all() or not key_padding_mask.all()
            or b_q.any() or b_k.any()):
        return _numpy_fallback(query, key, value, attn_mask, key_padding_mask,
                               w_q, b_q, w_k, b_k, w_v, b_v, w_o, b_o)

    nc = build()
    in_maps = _host_prep(query, key, value, w_q, w_k, w_v, w_o)
    res = run_bass_kernel_spmd(nc, in_maps, list(range(NCORES)))

    out = np.empty((T, D), np.float32)
    for c in range(NCORES):
        out[TOK_PER_CORE * c:TOK_PER_CORE * (c + 1)] = \
            res.results[c]["out"].reshape(TOK_PER_CORE, D)
    # exact host-side bias fold: softmax rows sum to 1 => + (b_v @ w_o.T + b_o)
    out += b_v @ w_o.T + b_o
    return out.reshape(B, S, D)
